# revision 22
# baseline (speedup 1.0000x reference)
"""AverageSpanExtractor Trainium2 kernel.

Math: out[b, n, :] = mean(seq[b, start_n:end_n, :]) * mask[b, n]

Strategy (per core; data-parallel over batch across 8 cores):
  1. Load seq [S=2048, D=512] f32 (chunks split across the SP and ACT HWDGE
     queues), cast fp16 into xf (stripe layout: token p at partition p%128,
     stripe p//128). Per-block grand totals via ones-matmul during the load
     (feeds the offset table early).
  2. Build a RECENTERED exclusive-prefix table in SBUF (fp16, same layout):
       pos p holds  T[p] = E[p+1] - (p+1)*mu,
     mu = E[2048]/2048. Recentering turns the prefix walk into a bridge
     (|T| <~ 160), which makes fp16 storage viable. pos 2047 is zeroed and
     aliases E[0] (start==0 maps there via idx = (r - 1) mod 2048).
     Phase 1b per block: PE accumulates u_tri.T @ x_b (in-block cumsum) and
     a selector matmul (fp16 hi/lo block offset + -(p+1)*mu via an iota row)
     into ONE PSUM bank; ACT packs PSUM -> fp16 table (no DVE: 2-operand
     DVE ops run at half rate and were the old critical path).
  3. Gather 2048 rows (ends||starts per 256-span chunk) DIRECTLY FROM SBUF
     with gpsimd.dma_gather(transpose=True, sbuf_tokens_per_rank=128) — no
     DRAM round-trip. num_idxs=512 each (HW crashes above ~768).
  4. Width-1 spans take an exact path: two more prepared gathers read
     x[start] from xf, copy_predicated over d, with their add-back width
     zeroed.
  5. DVE subtract (ends - starts), PE transposes back to span-major with
     identity rhs + w*mu add-back in the same PSUM, ACT scales by
     mask/width, stores fp16; host upcasts to f32.
"""

import numpy as np

import concourse.bacc as bacc
import concourse.bass as bass
import concourse.tile as tile
from concourse import mybir
from concourse.bass import AP
from concourse.library_config import mlp
from concourse.masks import make_identity, make_upper_triangular
from concourse.tile_rust import add_dep_helper

# Problem shape (hardcoded per contract).
B, S, D, N = 8, 2048, 512, 1024
NBLK = S // 128          # 16 token blocks (= table stripes)
NTILE = N // 128         # 8 span tiles
NGATHER = 4              # main gathers (256 spans each)

F32 = mybir.dt.float32
I32 = mybir.dt.int32
I16 = mybir.dt.int16
F16 = mybir.dt.float16


def build_kernel_body(tc: tile.TileContext, seq: AP, spans: AP, maskw: AP,
                      out: AP, ctx, dbg=None, w1_patch=True, gq=(0, 1, 0, 1)):
    nc = tc.nc
    sbuf = ctx.enter_context(tc.tile_pool(name="sbuf", bufs=1))
    const = ctx.enter_context(tc.tile_pool(name="const", bufs=1))
    gpool = ctx.enter_context(tc.tile_pool(name="gpool", bufs=1))
    opool = ctx.enter_context(tc.tile_pool(name="opool", bufs=3))
    psum_t = ctx.enter_context(tc.tile_pool(name="pt", bufs=2, space="PSUM"))
    psum_b = ctx.enter_context(tc.tile_pool(name="pb", bufs=3, space="PSUM"))
    psum_o = ctx.enter_context(tc.tile_pool(name="po", bufs=2, space="PSUM"))
    psum_off = ctx.enter_context(tc.tile_pool(name="poff", bufs=1, space="PSUM"))

    # ---------------- constants (Pool builds f32, DVE casts fp16) ----------
    u_tri_f = const.tile([128, 128], F32, tag="u_tri_f")
    make_upper_triangular(nc, u_tri_f[:], val=1.0, diag=True)
    u16s_f = const.tile([16, 16], F32, tag="u16s_f")
    make_upper_triangular(nc, u16s_f[:], val=1.0, diag=False)
    ident_f = const.tile([128, 128], F32, tag="ident_f")
    make_identity(nc, ident_f[:])
    zrow16 = const.tile([1, D], F16, tag="zrow16")
    nc.vector.memset(zrow16[:], 0.0)
    ones128 = const.tile([128, 1], F16, tag="ones128")
    nc.vector.memset(ones128[:], 1.0)

    # selbig [65, 2048] fp16; per block b, cols 128b:128b+128 form the
    # offset-matmul lhsT: row b = 1 (hi(off_b)), row 32+b = 1 (lo(off_b)),
    # row 64 = -(128b + m + 1) (iota; multiplies the mu row of o2e).
    # 32-row spacing: gpsimd affine_select needs 32-aligned start partitions.
    selbig_f = const.tile([65, 2048], F32, tag="selbig_f")
    nc.gpsimd.memset(selbig_f[:], 0.0)
    for base_row in (0, 32):
        nc.gpsimd.affine_select(
            out=selbig_f[base_row:base_row + 16, :],
            in_=selbig_f[base_row:base_row + 16, :],
            compare_op=mybir.AluOpType.not_equal, fill=1.0,
            base=0, pattern=[[-1, 16], [0, 128]], channel_multiplier=1)
    jrow_i = const.tile([1, 2048], I16, tag="jrow_i")
    nc.gpsimd.iota(jrow_i[:], pattern=[[-128, 16], [-1, 128]], base=-1,
                   channel_multiplier=0)

    u_tri = const.tile([128, 128], F16, tag="u_tri")
    nc.vector.tensor_copy(u_tri[:], u_tri_f[:])
    u16s = const.tile([16, 16], F16, tag="u16s")
    nc.vector.tensor_copy(u16s[:], u16s_f[:])
    ident = const.tile([128, 128], F16, tag="ident")
    nc.vector.tensor_copy(ident[:], ident_f[:])
    selbig = const.tile([65, 2048], F16, tag="selbig")
    nc.vector.tensor_copy(selbig[:], selbig_f[:])
    nc.vector.tensor_copy(selbig[64:65, :], jrow_i[:])

    # ---------------- index / scale staging (Sync queue, before seq) -------
    # Main gather t covers spans 256t..256t+255 with 512 idxs:
    #   i in [0,256):   end of span 256t + i;  [256,512): start of same
    # dma_gather reads idxs[p, c] = list[c*16 + p%16] -> position i sits at
    # column i//16, partition i%16 -> global column c = 32t + i//16.
    a32 = sbuf.tile([16, 128], I32, tag="a32")
    for t in range(NGATHER):
        nc.sync.dma_start(
            a32[:, 32 * t:32 * t + 16],
            AP(spans.tensor, 512 * t + 1, [[2, 16], [32, 16]]))
        nc.sync.dma_start(
            a32[:, 32 * t + 16:32 * t + 32],
            AP(spans.tensor, 512 * t, [[2, 16], [32, 16]]))

    e_row = sbuf.tile([1, N], I32, tag="e_row")
    s_row = sbuf.tile([1, N], I32, tag="s_row")
    nc.sync.dma_start(e_row[:], AP(spans.tensor, 1, [[0, 1], [2, N]]))
    nc.sync.dma_start(s_row[:], AP(spans.tensor, 0, [[0, 1], [2, N]]))

    # per-span scale = mask / width, laid out [p, j] for span n = j*128 + p
    st_pj = sbuf.tile([128, NTILE], I32, tag="st_pj")
    en_pj = sbuf.tile([128, NTILE], I32, tag="en_pj")
    mk_pj = sbuf.tile([128, NTILE], I32, tag="mk_pj")
    nc.sync.dma_start(st_pj[:], AP(spans.tensor, 0, [[2, 128], [256, NTILE]]))
    nc.sync.dma_start(en_pj[:], AP(spans.tensor, 1, [[2, 128], [256, NTILE]]))
    nc.sync.dma_start(mk_pj[:], AP(maskw.tensor, 0, [[1, 128], [128, NTILE]]))

    # Table position p holds E[p+1]; gather index = (r - 1) mod 2048 so that
    # start==0 hits pos 2047, which is explicitly zeroed (E[0] = 0).
    # idx = r - 1 + 2048*(r == 0), via is_equal (bitwise ops don't lower).
    eq0 = sbuf.tile([16, 128], I32, tag="eq0")
    nc.vector.tensor_scalar(out=eq0[:], in0=a32[:], scalar1=0, scalar2=None,
                            op0=mybir.AluOpType.is_equal)
    t1 = sbuf.tile([16, 128], I32, tag="t1")
    nc.vector.tensor_scalar(out=t1[:], in0=eq0[:], scalar1=2048, scalar2=-1,
                            op0=mybir.AluOpType.mult,
                            op1=mybir.AluOpType.add)
    am = sbuf.tile([16, 128], I32, tag="am")
    nc.vector.tensor_tensor(out=am[:], in0=a32[:], in1=t1[:],
                            op=mybir.AluOpType.add)
    # idx16 cols 0:128 = main (remapped); cols 128:192 = raw starts for the
    # w==1 xf gathers (32 cols per 512-span half).
    idx16 = sbuf.tile([128, 192], I16, tag="idx16")
    nc.vector.tensor_copy(idx16[0:16, 0:128], am[:])
    for u in range(2):
        for h in range(2):
            nc.vector.tensor_copy(
                idx16[0:16, 128 + 32 * u + 16 * h:144 + 32 * u + 16 * h],
                a32[:, 64 * u + 32 * h + 16:64 * u + 32 * h + 32])
    # replicate 16-partition wrap across all 128 partitions (8 Q7 cores)
    nc.scalar.dma_start(idx16[16:32, :], idx16[0:16, :])
    nc.scalar.dma_start(idx16[32:64, :], idx16[0:32, :])
    nc.scalar.dma_start(idx16[64:128, :], idx16[0:64, :])

    # ------- gather ucode library + prepared SBUF-source main gathers ------
    # Main preps are traced while the table is still unwritten so they carry
    # no RAW deps (desc-gen reads only idx16); data safety comes from the
    # triggers' explicit deps on the pack instructions. The xf gathers are
    # prepped LATER (after the casts) so the casts don't WAR-wait on them.
    table = sbuf.tile([128, NBLK, D], F16, tag="table")
    xf = sbuf.tile([128, NBLK, D], F16, tag="xf")
    nc.gpsimd.load_library(mlp)
    gsems = [ctx.enter_context(nc.semaphore(f"gsem{g}"))
             for g in range(NGATHER)]
    gts = []
    for g in range(NGATHER):
        g_t = gpool.tile([128, 4, 512], F16, tag=f"g{g}")
        nc.gpsimd.dma_gather(
            out_ap=g_t[:], in_ap=table[:],
            idxs_ap=idx16[:, 32 * g:32 * g + 32],
            num_idxs=512, num_idxs_reg=512, elem_size=D,
            transpose=True, sbuf_tokens_per_rank=128,
            sbuf_free_dim_per_rank=D * 2,
            prepare_only=True, sem=gsems[g], queue_num=gq[g])
        gts.append(g_t)

    # ------- phase 1a: seq loads (SP+ACT HWDGE), cast fp16 on DVE, ---------
    # ------- per-block grand totals via ones-matmul into t16 ---------------
    # matmul PSUM outs must start at partition 0/32/64, and DMA cannot read
    # PSUM — so stage each block total at a free-dim slot of a partition-0
    # row (ACT copy), then unwrap to [16, D] with one DMA.
    xpool = ctx.enter_context(tc.tile_pool(name="xpool", bufs=2))
    stage16 = sbuf.tile([1, NBLK, D], F32, tag="stage16")
    cast_insts = []
    for q in range(NBLK // 2):
        sl = (slice(None), slice(2 * q, 2 * q + 2), slice(None))
        xs = xpool.tile([128, 2, D], F32, tag="xs")
        dma_eng = nc.sync if q % 2 == 0 else nc.scalar
        dma_eng.dma_start(
            xs[:],
            seq[256 * q:256 * (q + 1), :].rearrange("(j p) d -> p j d", p=128))
        cast_insts.append(nc.vector.tensor_copy(xf[sl], xs[:]))
        for b in (2 * q, 2 * q + 1):
            tp = psum_t.tile([1, D], F32, tag="tp")
            nc.tensor.matmul(out=tp[:], lhsT=ones128[:],
                             rhs=xf[:, b, :], start=True, stop=True)
            nc.scalar.copy(stage16[:, b, :], tp[:])
    t16 = sbuf.tile([16, D], F32, tag="t16")
    nc.scalar.dma_start(t16[:], stage16[:])

    # w / scale chains, traced after the casts so they don't delay them on
    # the DVE queue; two i32 scratch rows reused in place.
    nc.vector.tensor_tensor(out=e_row[:], in0=e_row[:], in1=s_row[:],
                            op=mybir.AluOpType.subtract)     # e_row := w
    nc.vector.tensor_scalar(out=s_row[:], in0=e_row[:], scalar1=1,
                            scalar2=None,
                            op0=mybir.AluOpType.is_equal)    # s_row := w==1
    w1f = sbuf.tile([1, N], I16, tag="w1f")
    nc.vector.tensor_copy(w1f[:], s_row[:])
    nc.vector.tensor_scalar(out=s_row[:], in0=s_row[:], scalar1=-1, scalar2=1,
                            op0=mybir.AluOpType.mult,
                            op1=mybir.AluOpType.add)         # s_row := w!=1
    nc.vector.tensor_tensor(out=e_row[:], in0=e_row[:], in1=s_row[:],
                            op=mybir.AluOpType.mult)         # e_row := w*(w!=1)
    w16 = sbuf.tile([1, N], F16, tag="w16")
    nc.vector.tensor_copy(w16[:], e_row[:])

    w_i = sbuf.tile([128, NTILE], I32, tag="w_i")
    nc.vector.tensor_tensor(out=w_i[:], in0=en_pj[:], in1=st_pj[:],
                            op=mybir.AluOpType.subtract)
    w_f = sbuf.tile([128, NTILE], F32, tag="w_f")
    nc.vector.tensor_copy(w_f[:], w_i[:])
    r_f = sbuf.tile([128, NTILE], F32, tag="r_f")
    nc.vector.reciprocal(r_f[:], w_f[:])
    m_f = sbuf.tile([128, NTILE], F32, tag="m_f")
    nc.vector.tensor_copy(m_f[:], mk_pj[:])
    scale = sbuf.tile([128, NTILE], F32, tag="scale")
    nc.vector.tensor_tensor(out=scale[:], in0=r_f[:], in1=m_f[:],
                            op=mybir.AluOpType.mult)

    # ------- offsets: Off = strict_upper(U16).T @ totals (fp16 hi/lo), -----
    # ------- mu = grand_total / 2048 ---------------------------------------
    th = sbuf.tile([16, D], F16, tag="th")
    nc.vector.tensor_copy(th[:], t16[:])
    tl = sbuf.tile([16, D], F16, tag="tl")
    nc.vector.tensor_tensor(out=tl[:], in0=t16[:], in1=th[:],
                            op=mybir.AluOpType.subtract)
    poff = psum_off.tile([16, D], F32, tag="poff")
    nc.tensor.matmul(out=poff[:], lhsT=u16s[:], rhs=th[:], start=True, stop=False)
    nc.tensor.matmul(out=poff[:], lhsT=u16s[:], rhs=tl[:], start=False, stop=True)
    off16 = sbuf.tile([16, D], F32, tag="off16")
    nc.vector.tensor_copy(off16[:], poff[:])

    # grand total = off16[15] + t16[15]; engines need partition-0 APs, so
    # DMA those two rows down to partition 0 first.
    t15 = sbuf.tile([1, D], F32, tag="t15")
    nc.scalar.dma_start(t15[:], t16[15:16, :])
    o15 = sbuf.tile([1, D], F32, tag="o15")
    nc.scalar.dma_start(o15[:], off16[15:16, :])
    tot = sbuf.tile([1, D], F32, tag="tot")
    nc.vector.tensor_tensor(out=tot[:], in0=o15[:], in1=t15[:],
                            op=mybir.AluOpType.add)
    mu32 = sbuf.tile([1, D], F32, tag="mu32")
    nc.vector.tensor_scalar(out=mu32[:], in0=tot[:], scalar1=1.0 / S,
                            scalar2=None, op0=mybir.AluOpType.mult)
    mu16 = sbuf.tile([1, D], F16, tag="mu16")
    nc.vector.tensor_copy(mu16[:], mu32[:])

    # o2e [65, 512] fp16: rows 0:16 hi(off), 32:48 lo(off), row 64 mu.
    # NB: engine APs must start at partition 0/32/64/96; a single
    # 65-partition memset wedges the DVE, so split it.
    o2e = sbuf.tile([65, D], F16, tag="o2e")
    nc.vector.memset(o2e[0:64, :], 0.0)
    nc.vector.memset(o2e[64:65, :], 0.0)
    nc.vector.tensor_copy(o2e[0:16, :], off16[:])
    nc.vector.tensor_tensor(out=o2e[32:48, :], in0=off16[:], in1=o2e[0:16, :],
                            op=mybir.AluOpType.subtract)
    nc.vector.tensor_copy(o2e[64:65, :], mu16[:])

    # ---------------- phase 1b: cumsum + offsets in one PSUM, ACT pack -----
    pack_insts = []
    for b in range(NBLK):
        pb = psum_b.tile([128, D], F32, tag="pb")
        nc.tensor.matmul(out=pb[:], lhsT=u_tri[:], rhs=xf[:, b, :],
                         start=True, stop=False)
        nc.tensor.matmul(out=pb[:], lhsT=selbig[:, 128 * b:128 * (b + 1)],
                         rhs=o2e[:], start=False, stop=True)
        pk = nc.scalar.copy(table[:, b, :], pb[:])
        pack_insts.append(pk)
    # pos 2047 aliases E[0] = 0 (never a valid end; start==0 maps here);
    # engines can't address partition 127, so zero it via DMA.
    zfix = nc.scalar.dma_start(table[127:128, NBLK - 1, :], zrow16[:])
    pack_insts.append(zfix)

    if dbg is not None:
        nc.sync.dma_start(dbg["tbl"][:], table[:])
        nc.sync.dma_start(dbg["idx"][:], idx16[:])
        nc.sync.dma_start(dbg["scale"][:], scale[:])
        nc.sync.dma_start(dbg["mu"][:], mu32[:])
        nc.sync.dma_start(dbg["off16"][:], off16[:])

    # ---------------- phase 2: fire main gathers, combine ------------------
    trig2 = nc.gpsimd.trigger_dma(count=None, queue_num=0)
    trig2b = nc.gpsimd.trigger_dma(count=None, queue_num=1) if 1 in gq else None
    for pk in pack_insts:
        add_dep_helper(trig2.ins, pk.ins, sync=True,
                       reason="main gathers read SBUF table")
        if trig2b is not None:
            add_dep_helper(trig2b.ins, pk.ins, sync=True,
                           reason="main gathers read SBUF table")
    trigof = {g: (trig2 if gq[g] == 0 else trig2b) for g in range(NGATHER)}

    # xf gathers for the w==1 patch: prepped after the casts (RAW dep), so
    # they never stall the casts; fired right after the main triggers.
    trig1 = None
    xsems = [ctx.enter_context(nc.semaphore(f"xsem{u}")) for u in range(2)]
    xgs = []
    if w1_patch:
        for u in range(2):
            xg_u = gpool.tile([128, 4, 512], F16, tag=f"xg{u}")
            nc.gpsimd.dma_gather(
                out_ap=xg_u[:], in_ap=xf[:],
                idxs_ap=idx16[:, 128 + 32 * u:160 + 32 * u],
                num_idxs=512, num_idxs_reg=512, elem_size=D,
                transpose=True, sbuf_tokens_per_rank=128,
                sbuf_free_dim_per_rank=D * 2,
                prepare_only=True, sem=xsems[u], queue_num=0)
            xgs.append(xg_u)
        trig1 = nc.gpsimd.trigger_dma(count=None, queue_num=0)
        # wb[p, n] = (w_n == 1) broadcast to all partitions (copy_predicated)
        wb = sbuf.tile([128, N], I16, tag="wb")
        nc.gpsimd.partition_broadcast(wb[:], w1f[:])

    for g in range(NGATHER):
        g_t = gts[g]
        u, h = g // 2, g % 2
        if dbg is not None:
            gd = nc.sync.dma_start(dbg["g"][:, 4 * g:4 * g + 4, :], g_t[:])
            gd._wait_ge(gsems[g], 16)
            add_dep_helper(gd.ins, trigof[g].ins, sync=False,
                           reason="consume after trigger")
            if w1_patch and h == 0:
                xd = nc.sync.dma_start(dbg["xg"][:, 4 * u:4 * u + 4, :],
                                       xgs[u][:])
                xd._wait_ge(xsems[u], 16)
                add_dep_helper(xd.ins, trig1.ins, sync=False,
                               reason="consume after trigger")
        # ends - starts in transposed (feature-major) layout, in place over
        # the ends half of the gather tile (elementwise, no cross-elem deps)
        d_t = g_t
        tt = nc.vector.tensor_tensor(out=d_t[:, :, 0:256], in0=g_t[:, :, 0:256],
                                     in1=g_t[:, :, 256:512],
                                     op=mybir.AluOpType.subtract)
        tt._wait_ge(gsems[g], 16)
        add_dep_helper(tt.ins, trigof[g].ins, sync=False,
                       reason="consume after trigger")
        # patch w==1 spans with exact x[start] (their add-back width is 0)
        if w1_patch:
            for c in range(4):
                bl = nc.vector.copy_predicated(
                    out=d_t[:, c, 0:256], mask=wb[:, 256 * g:256 * (g + 1)],
                    data=xgs[u][:, c, 256 * h:256 * (h + 1)])
                if c == 0:
                    bl._wait_ge(xsems[u], 16)
                    add_dep_helper(bl.ins, trig1.ins, sync=False,
                                   reason="consume after trigger")
        for k in range(2):
            j = 2 * g + k
            po = psum_o.tile([128, D], F32, tag="po")
            for c in range(4):
                # transpose 128x128 back to span-major ...
                nc.tensor.matmul(out=po[:, 128 * c:128 * (c + 1)],
                                 lhsT=d_t[:, c, 128 * k:128 * (k + 1)],
                                 rhs=ident[:], start=True, stop=False)
                # ... and add back w_n * mu (undo recentering), same PSUM
                nc.tensor.matmul(out=po[:, 128 * c:128 * (c + 1)],
                                 lhsT=w16[0:1, 128 * j:128 * (j + 1)],
                                 rhs=mu16[:, 128 * c:128 * (c + 1)],
                                 start=False, stop=True)
            o_t = opool.tile([128, D], F16, tag="o")
            nc.scalar.mul(o_t[:], po[:], scale[:, j:j + 1])
            nc.sync.dma_start(out[128 * j:128 * (j + 1), :], o_t[:])


def build_nc(debug_taps=False, w1_patch=True, gq=(0, 1, 0, 1)):
    nc = bacc.Bacc("TRN2", target_bir_lowering=False, debug=False,
                   dynamic_dma_scratch_size=2 ** 16, num_swdge_queues=2)
    seq = nc.dram_tensor("seq", [S, D], F32, kind="ExternalInput")
    spans = nc.dram_tensor("spans", [N, 2], I32, kind="ExternalInput")
    maskw = nc.dram_tensor("maskw", [N], I32, kind="ExternalInput")
    out = nc.dram_tensor("out", [N, D], F16, kind="ExternalOutput")
    dbg = None
    if debug_taps:
        dbg = {
            "tbl": nc.dram_tensor("dbg_tbl", [128, NBLK, D], F16,
                                  kind="ExternalOutput").ap(),
            "idx": nc.dram_tensor("dbg_idx", [128, 192], I16,
                                  kind="ExternalOutput").ap(),
            "scale": nc.dram_tensor("dbg_scale", [128, NTILE], F32,
                                    kind="ExternalOutput").ap(),
            "g": nc.dram_tensor("dbg_g", [128, 4 * NGATHER, 512], F16,
                                kind="ExternalOutput").ap(),
            "xg": nc.dram_tensor("dbg_xg", [128, 8, 512], F16,
                                 kind="ExternalOutput").ap(),
            "mu": nc.dram_tensor("dbg_mu", [1, D], F32,
                                 kind="ExternalOutput").ap(),
            "off16": nc.dram_tensor("dbg_off16", [16, D], F32,
                                    kind="ExternalOutput").ap(),
        }
    from contextlib import ExitStack
    with tile.TileContext(nc) as tc:
        with ExitStack() as ctx:
            build_kernel_body(tc, seq.ap(), spans.ap(), maskw.ap(), out.ap(),
                              ctx, dbg=dbg, w1_patch=w1_patch, gq=gq)
    nc.compile()
    return nc


_NC_CACHE = None


def kernel(sequence_tensor: np.ndarray, span_indices: np.ndarray,
           span_indices_mask: np.ndarray) -> np.ndarray:
    global _NC_CACHE
    from concourse.bass_utils import run_bass_kernel_spmd

    if _NC_CACHE is None:
        _NC_CACHE = build_nc()
    nc = _NC_CACHE

    spans_i32 = np.ascontiguousarray(np.asarray(span_indices).astype(np.int32))
    mask_i32 = np.ascontiguousarray(np.asarray(span_indices_mask).astype(np.int32))
    seq_f32 = np.ascontiguousarray(sequence_tensor, dtype=np.float32)

    in_maps = [
        {"seq": seq_f32[b], "spans": spans_i32[b], "maskw": mask_i32[b]}
        for b in range(B)
    ]
    res = run_bass_kernel_spmd(nc, in_maps, core_ids=list(range(B)))
    return np.stack([r["out"] for r in res.results], axis=0).astype(np.float32)
